# revision 2
# baseline (speedup 1.0000x reference)
"""Trainium2 Bass kernel for the Hodge-Laplacian GNN encoder (nn_Encoder_71811853189566).

Math (reference): h = relu(x@W0 + (B1^T B1 x)@W1 + (B2 B2^T x)@W2);
out[g] = mean_{e: edge_batch[e]==g} h[e]; returns (out, out, out).

v2 strategy (wire-optimized): the previous version baked signed gather
tables [P, W*D] on the host and shipped ~130MB/core over the axon tunnel
(dispatch wall ~18s). Now each core receives only its 1/8 x-shard in
block-permuted order (bf16, 8MB) plus int32 index tables (~4MB); on device
we AllGather the x shards into a full table in DRAM, append a negated copy
(sign trick), then fetch each edge's signed pair rows with SWDGE indirect
DMA. Per-block compute: DVE reduces the K gathered chunks, PE transposes
[x | lower] jointly (one [P,128] transpose), applies the stacked weights
[W0+2W1; W1] and W2 into PSUM, ACT applies relu, and a one-hot matmul
accumulates the graph readout in a persistent PSUM tile. Host sums the 8
per-core [G, D] partials and divides by graph counts.
"""

import math
import numpy as np

# ---------------- problem constants (hardcoded per contract) ----------------
N_NODES = 200_000
N_EDGES = 500_000
N_TRI = 250_000
D = 64
G = 128
N_CORES = 8
P = 128

CAP_LO = 192   # max gather-tile width (64-elem chunks) for lower groups
CAP_UP = 96    # same for upper groups
XGROUP = 16    # x-tile blocks per direct load
NEG_CR = 8192  # rows per negation chunk
GCH = 1        # index columns per indirect-DMA instruction (HW honors one
               # index per out-AP partition row; see probe_gather7)


# ---------------- host-side index prep ----------------

def _csr(keys, n):
    order = np.argsort(keys, kind="stable")
    ptr = np.searchsorted(keys[order], np.arange(n + 1))
    return order, ptr


def _expand(e_ptr, e_order, mid_key, vals, m_ptr, m_order, tgt_key, m_vals, n_edges):
    e_rep = np.repeat(np.arange(n_edges, dtype=np.int64), e_ptr[1:] - e_ptr[:-1])
    j1 = e_order
    m = mid_key[j1]
    s1 = vals[j1]
    cnt2 = (m_ptr[m + 1] - m_ptr[m]).astype(np.int64)
    off = np.concatenate(([0], np.cumsum(cnt2)))
    idx_in_run = np.arange(off[-1], dtype=np.int64) - np.repeat(off[:-1], cnt2)
    j2 = m_order[np.repeat(m_ptr[m], cnt2) + idx_in_run]
    pair_e = np.repeat(e_rep, cnt2)
    pair_e2 = tgt_key[j2]
    pair_sign = np.repeat(s1, cnt2) * m_vals[j2]
    pair_ptr = np.searchsorted(pair_e, np.arange(n_edges + 1))
    return pair_ptr, pair_e2.astype(np.int64), pair_sign.astype(np.float32)


def build_pairs(n_nodes, n_edges, n_tri, b1_rows, b1_cols, b1_vals,
                b2_rows, b2_cols, b2_vals):
    b1_rows = np.asarray(b1_rows, np.int64); b1_cols = np.asarray(b1_cols, np.int64)
    b1_vals = np.asarray(b1_vals, np.float32)
    b2_rows = np.asarray(b2_rows, np.int64); b2_cols = np.asarray(b2_cols, np.int64)
    b2_vals = np.asarray(b2_vals, np.float32)

    e_order, e_ptr = _csr(b1_cols, n_edges)
    n_order, n_ptr = _csr(b1_rows, n_nodes)
    lo_ptr, lo_e2, lo_sign = _expand(e_ptr, e_order, b1_rows, b1_vals,
                                     n_ptr, n_order, b1_cols, b1_vals, n_edges)

    # remove self pairs; device adds 2*x[e]@W1 globally (W0' fold);
    # edges whose removed self-sign-sum sigma != 2 get (e, -1/+1) compensation.
    own = np.repeat(np.arange(n_edges, dtype=np.int64), lo_ptr[1:] - lo_ptr[:-1])
    is_self = lo_e2 == own
    sigma = np.zeros(n_edges, np.float64)
    np.add.at(sigma, own[is_self], lo_sign[is_self].astype(np.float64))
    keep = ~is_self
    cnt = np.bincount(own[keep], minlength=n_edges).astype(np.int64)
    lo_e2 = lo_e2[keep]; lo_sign = lo_sign[keep]
    # compensation pairs
    delta = np.rint(sigma - 2.0).astype(np.int64)
    bad = np.nonzero(delta)[0]
    if len(bad):
        comp_e = np.repeat(bad, np.abs(delta[bad]))
        comp_s = np.repeat(np.sign(delta[bad]).astype(np.float32), np.abs(delta[bad]))
        all_e = np.concatenate([own[keep], comp_e])
        order = np.argsort(all_e, kind="stable")
        lo_e2 = np.concatenate([lo_e2, comp_e])[order]
        lo_sign = np.concatenate([lo_sign, comp_s])[order]
        cnt += np.bincount(comp_e, minlength=n_edges).astype(np.int64)
    lo_ptr = np.concatenate(([0], np.cumsum(cnt)))

    ue_order, ue_ptr = _csr(b2_rows, n_edges)
    t_order, t_ptr = _csr(b2_cols, n_tri)
    up_ptr, up_e2, up_sign = _expand(ue_ptr, ue_order, b2_cols, b2_vals,
                                     t_ptr, t_order, b2_rows, b2_vals, n_edges)
    return lo_ptr, lo_e2, lo_sign, up_ptr, up_e2, up_sign


def _pack_groups(K, cap):
    """Greedy pack consecutive blocks into groups with sum(K) <= cap (min 1 block).
    Returns (group_of_block, group_starts, group_widths, block_off_in_group)."""
    gob, starts, widths, boff = [], [], [], []
    cur_w, cur_g = 0, -1
    for b, k in enumerate(K):
        k = int(k)
        if cur_g < 0 or (cur_w + k > cap and cur_w > 0):
            cur_g += 1
            starts.append(b)
            widths.append(0)
            cur_w = 0
        gob.append(cur_g)
        boff.append(cur_w)
        widths[cur_g] = cur_w + k
        cur_w += k
    return gob, starts, widths, boff


class Plan:
    pass


def make_plan(n_edges, n_cores, lo_ptr, up_ptr):
    """Cross-core program plan + per-core permutations + slot table layout."""
    pl = Plan()
    Ec = n_edges // n_cores
    NB = math.ceil(Ec / P)
    NBP = NB * P
    pl.Ec, pl.NB, pl.NBP = Ec, NB, NBP
    # Device table: AllGather of per-core [xin; -xin] blocks, so rank c's
    # rows occupy [2*NBP*c, 2*NBP*(c+1)): positive slot s at 2*NBP*c + s,
    # negated at 2*NBP*c + NBP + s.
    pl.NEG_OFF = NBP
    pl.TAB_ROWS = 2 * n_cores * NBP
    pl.ZERO_ROW = Ec                  # core 0's first dummy slot (zeros)
    klo_all = (lo_ptr[1:] - lo_ptr[:-1]).astype(np.int64)
    kup_all = (up_ptr[1:] - up_ptr[:-1]).astype(np.int64)
    pl.perms = []          # per-core: global edge id per local slot (-1 = dummy)
    pos = np.empty(n_edges, np.int64)  # edge -> table row
    Klo_cb = np.zeros((n_cores, NB), np.int64)
    Kup_cb = np.zeros((n_cores, NB), np.int64)
    for c in range(n_cores):
        eg = np.arange(c * Ec, (c + 1) * Ec, dtype=np.int64)
        order = np.lexsort((-klo_all[eg], -kup_all[eg]))
        perm = np.full(NBP, -1, np.int64)
        perm[:Ec] = eg[order]
        pl.perms.append(perm)
        pos[perm[:Ec]] = c * 2 * NBP + np.arange(Ec)
        kl = np.zeros(NBP, np.int64); ku = np.zeros(NBP, np.int64)
        kl[:Ec] = klo_all[eg[order]]; ku[:Ec] = kup_all[eg[order]]
        Klo_cb[c] = kl.reshape(NB, P).max(axis=1)
        Kup_cb[c] = ku.reshape(NB, P).max(axis=1)
    pl.pos = pos
    pl.K_LO = Klo_cb.max(axis=0)
    pl.K_UP = Kup_cb.max(axis=0)
    pl.lgr = _pack_groups(pl.K_LO, CAP_LO)
    pl.ugr = _pack_groups(pl.K_UP, CAP_UP)
    pl.Wl = int(pl.K_LO.sum()) + (int(pl.K_LO.sum()) & 1)   # even for nibble pack
    pl.Wu = int(pl.K_UP.sum()) + (int(pl.K_UP.sum()) & 1)
    lo_goff = np.concatenate(([0], np.cumsum(pl.lgr[2])))
    up_goff = np.concatenate(([0], np.cumsum(pl.ugr[2])))
    pl.lo_bcol = np.array([lo_goff[pl.lgr[0][b]] + pl.lgr[3][b] for b in range(NB)])
    pl.up_bcol = np.array([up_goff[pl.ugr[0][b]] + pl.ugr[3][b] for b in range(NB)])
    pl.lo_goff = lo_goff
    pl.up_goff = up_goff
    # meta blob layout (bytes per partition); Wl/Wu are even
    pl.OFF_LLO = 0
    pl.OFF_ULO = pl.OFF_LLO + 2 * pl.Wl
    pl.OFF_LNIB = pl.OFF_ULO + 2 * pl.Wu
    pl.OFF_UNIB = pl.OFF_LNIB + pl.Wl // 2
    pl.OFF_BATCH = pl.OFF_UNIB + pl.Wu // 2
    pl.META = -(-(pl.OFF_BATCH + NB) // 4) * 4
    return pl


def _fill_idx(pl, perm, pair_ptr, pair_e2, pair_sign, bcol, Wtot, NB):
    """Build [P, Wtot] int32 gather-index array for one core.

    Index value = pos[e2] (positive pair) or pos[e2] + NEG_OFF (negative);
    padding points at ZERO_ROW (a zeroed dummy slot)."""
    arr = np.full((P, Wtot), pl.ZERO_ROW, np.int32)
    slots = np.arange(NB * P, dtype=np.int64)
    real = perm >= 0
    e = perm[real]
    k = (pair_ptr[e + 1] - pair_ptr[e]).astype(np.int64)
    srows = (slots[real] % P)
    sb = slots[real] // P
    base = srows * Wtot + bcol[sb]
    dest = np.repeat(base, k) + (np.arange(k.sum(), dtype=np.int64)
                                 - np.repeat(np.concatenate(([0], np.cumsum(k)))[:-1], k))
    off = np.concatenate(([0], np.cumsum(k)))
    src = np.repeat(pair_ptr[e], k) + (np.arange(k.sum(), dtype=np.int64)
                                       - np.repeat(off[:-1], k))
    vals = pl.pos[pair_e2[src]] + (pair_sign[src] < 0) * pl.NEG_OFF
    arr.flat[dest] = vals.astype(np.int32)
    return arr


def build_core_inputs(pl, c, edge_batch,
                      lo_ptr, lo_e2, lo_sign, up_ptr, up_e2, up_sign):
    perm = pl.perms[c]
    NB, NBP = pl.NB, pl.NBP
    real = perm >= 0
    bf = np.full(NBP, -1.0, np.float32)   # dummy slots never match any graph id
    bf[real] = edge_batch[perm[real]].astype(np.float32)
    batchf = np.ascontiguousarray(bf.reshape(NB, P).T)  # [P, NB]
    lidx = _fill_idx(pl, perm, lo_ptr, lo_e2, lo_sign, pl.lo_bcol, pl.Wl, NB)
    uidx = _fill_idx(pl, perm, up_ptr, up_e2, up_sign, pl.up_bcol, pl.Wu, NB)
    return dict(batchf=batchf, lidx=lidx, uidx=uidx)


# ---------------- bass program ----------------

def build_program(pl, xdt_name="float8e4", cdt_name="bfloat16"):
    import concourse.bacc as bacc
    import concourse.bass as bass
    import concourse.mybir as mybir
    import concourse.tile as tile
    from concourse.masks import make_identity

    f32 = mybir.dt.float32
    i32 = mybir.dt.int32
    u16 = mybir.dt.uint16
    u8 = mybir.dt.uint8
    xdt = getattr(mybir.dt, xdt_name)   # x shards + gather tiles
    cdt = getattr(mybir.dt, cdt_name)   # compute tiles
    NB = pl.NB
    NBP = pl.NBP
    AF = mybir.ActivationFunctionType
    ALU = mybir.AluOpType
    IOoA = bass.IndirectOffsetOnAxis

    nc = bacc.Bacc("TRN2", target_bir_lowering=False, debug=False)
    xin_d = nc.dram_tensor("xin", [NBP, D], xdt, kind="ExternalInput")
    meta_d = nc.dram_tensor("meta", [P, pl.META], u8, kind="ExternalInput")
    w012_d = nc.dram_tensor("w012", [3 * D, D], cdt, kind="ExternalInput")
    out_d = nc.dram_tensor("out", [P, D], f32, kind="ExternalOutput")

    lgob, lgst, lgw, _ = pl.lgr
    ugob, ugst, ugw, _ = pl.ugr
    max_lw = max(lgw); max_uw = max(ugw)

    def emit_gather(out_tile, idx_sb, goff, w, xsg):
        """Gather w columns of D-rows into out_tile[:, :w*D], one index
        column per indirect-DMA instruction. The out AP must stay 2-dim:
        the HW honors exactly one index per out-AP partition row, and 3-dim
        dest APs scramble the descriptor/offset pairing (probe_gather6)."""
        for j in range(w):
            nc.gpsimd.indirect_dma_start(
                out=out_tile[:, (j + 0) * D:(j + 1) * D],
                out_offset=None,
                in_=xsg[:],
                in_offset=IOoA(ap=idx_sb[:, goff + j:goff + j + 1], axis=0))

    with tile.TileContext(nc) as tc:
        with (
            tc.tile_pool(name="const", bufs=1) as cpool,
            tc.tile_pool(name="dram", bufs=1, space="DRAM") as dpool,
        ):
            # load the meta blob and unpack the 20-bit packed index tables
            # (u16 lows + nibble-packed highs) to i32, batch ids to f32
            meta_sb = cpool.tile([P, pl.META], u8)
            nc.sync.dma_start(meta_sb[:], meta_d[:])
            lidx_sb = cpool.tile([P, pl.Wl], i32)
            uidx_sb = cpool.tile([P, pl.Wu], i32)
            with tc.tile_pool(name="unpack", bufs=1) as upk:
                for sb, off_lo, off_nib, w in (
                        (lidx_sb, pl.OFF_LLO, pl.OFF_LNIB, pl.Wl),
                        (uidx_sb, pl.OFF_ULO, pl.OFF_UNIB, pl.Wu)):
                    lo_view = meta_sb[:, off_lo:off_lo + 2 * w].bitcast(u16)
                    nib_view = meta_sb[:, off_nib:off_nib + w // 2]
                    nib_i = upk.tile([P, pl.Wl // 2], i32, tag="tn")
                    nc.vector.tensor_copy(out=nib_i[:, : w // 2], in_=nib_view)
                    tmp = upk.tile([P, pl.Wl], i32, tag="tw")
                    ev = tmp[:, :w].rearrange("p (w2 two) -> p two w2", two=2)
                    nc.vector.tensor_scalar(
                        out=ev[:, 0, :], in0=nib_i[:, : w // 2],
                        scalar1=15, scalar2=None, op0=ALU.bitwise_and)
                    nc.vector.tensor_scalar(
                        out=ev[:, 1, :], in0=nib_i[:, : w // 2],
                        scalar1=4, scalar2=None, op0=ALU.logical_shift_right)
                    nc.vector.tensor_scalar(
                        out=tmp[:, :w], in0=tmp[:, :w],
                        scalar1=65536, scalar2=None, op0=ALU.mult)
                    nc.vector.tensor_copy(out=sb[:, :w], in_=lo_view)
                    nc.vector.tensor_tensor(
                        out=sb[:, :w], in0=sb[:, :w], in1=tmp[:, :w], op=ALU.add)

            batch = cpool.tile([P, NB], f32)
            nc.vector.tensor_copy(
                out=batch[:], in_=meta_sb[:, pl.OFF_BATCH:pl.OFF_BATCH + NB])
            w01 = cpool.tile([2 * D, D], cdt)
            nc.sync.dma_start(w01[:], w012_d[0:2 * D, :])
            w2 = cpool.tile([D, D], cdt)
            nc.sync.dma_start(w2[:], w012_d[2 * D:3 * D, :])
            iota_i = cpool.tile([P, P], i32)
            nc.gpsimd.iota(iota_i[:], pattern=[[1, P]], base=0,
                           channel_multiplier=0)
            iota = cpool.tile([P, P], f32)
            nc.vector.tensor_copy(out=iota[:], in_=iota_i[:])
            ident = cpool.tile([P, P], cdt)
            make_identity(nc, ident[:])

            xsg = dpool.tile([pl.TAB_ROWS, D], xdt)
            bounce = dpool.tile([2 * NBP, D], xdt)

            # ---- stage 1: build [xin; -xin] locally, then allgather ----
            nc.gpsimd.dma_start(bounce[0:NBP, :], xin_d[:])
            with tc.tile_pool(name="neg", bufs=2) as npool:
                r0 = 0
                while r0 < NBP:
                    cr = min(NEG_CR, NBP - r0)
                    n = cr // P
                    t_in = npool.tile([P, (NEG_CR // P) * D], xdt, tag="nin")
                    nc.sync.dma_start(
                        out=t_in[:, : n * D].rearrange("p (n d) -> p n d", n=n),
                        in_=xin_d[r0:r0 + cr, :].rearrange("(n p) d -> p n d", p=P))
                    t_out = npool.tile([P, (NEG_CR // P) * D], xdt, tag="nout")
                    nc.vector.tensor_scalar(
                        out=t_out[:, : n * D], in0=t_in[:, : n * D],
                        scalar1=-1.0, scalar2=None, op0=ALU.mult)
                    nc.sync.dma_start(
                        out=bounce[NBP + r0:NBP + r0 + cr, :]
                        .rearrange("(n p) d -> p n d", p=P),
                        in_=t_out[:, : n * D].rearrange("p (n d) -> p n d", n=n))
                    r0 += cr
            # Fence the collective from the bounce writers (cross-engine).
            tc.strict_bb_all_engine_barrier()
            nc.gpsimd.collective_compute(
                "AllGather",
                ALU.bypass,
                replica_groups=[list(range(N_CORES))],
                ins=[bounce[:].opt()],
                outs=[xsg[:].opt()],
            )

            # ---- stage 2: main block loop ----
            tc.strict_bb_all_engine_barrier()
            with (
                tc.tile_pool(name="lg", bufs=3) as lpool,
                tc.tile_pool(name="ug", bufs=3) as upool,
                tc.tile_pool(name="xg", bufs=3) as xpool,
                tc.tile_pool(name="wrk", bufs=4) as wpool,
                tc.tile_pool(name="psh", bufs=3, space="PSUM") as ph_pool,
                tc.tile_pool(name="pst", bufs=2, space="PSUM") as pt_pool,
                tc.tile_pool(name="psro", bufs=1, space="PSUM") as ro_pool,
            ):
                pro = ro_pool.tile([P, D], f32)

                lg_t = ug_t = xg_t = None
                cur_lg = cur_ug = cur_xg = None
                for b in range(NB):
                    if lgob[b] != cur_lg:
                        cur_lg = lgob[b]
                        lg_t = lpool.tile([P, max_lw * D], xdt, tag="lg")
                        emit_gather(lg_t, lidx_sb, int(pl.lo_goff[cur_lg]),
                                    lgw[cur_lg], xsg)
                    if ugob[b] != cur_ug:
                        cur_ug = ugob[b]
                        ug_t = upool.tile([P, max_uw * D], xdt, tag="ug")
                        emit_gather(ug_t, uidx_sb, int(pl.up_goff[cur_ug]),
                                    ugw[cur_ug], xsg)
                    if b // XGROUP != cur_xg:
                        cur_xg = b // XGROUP
                        nblk = min(XGROUP, NB - cur_xg * XGROUP)
                        xg_t = xpool.tile([P, XGROUP * D], xdt, tag="xg")
                        nc.sync.dma_start(
                            out=xg_t[:, : nblk * D]
                            .rearrange("p (n d) -> p n d", n=nblk),
                            in_=xin_d[cur_xg * XGROUP * P:
                                      (cur_xg * XGROUP + nblk) * P, :]
                            .rearrange("(n p) d -> p n d", p=P))

                    # -- per-block compute
                    Kl = int(pl.K_LO[b]); Ku = int(pl.K_UP[b])
                    lcol = int(pl.lo_bcol[b] - pl.lo_goff[lgob[b]])
                    ucol = int(pl.up_bcol[b] - pl.up_goff[ugob[b]])
                    xb = b - cur_xg * XGROUP

                    xl = wpool.tile([P, 2 * D], cdt, tag="xl")
                    nc.vector.tensor_copy(
                        out=xl[:, 0:D], in_=xg_t[:, xb * D:(xb + 1) * D])
                    with nc.allow_low_precision(reason="bf16 gather-sum tiles"):
                        if Kl == 0:
                            nc.vector.memset(xl[:, D:2 * D], 0.0)
                        elif Kl == 1:
                            nc.vector.tensor_copy(
                                out=xl[:, D:2 * D],
                                in_=lg_t[:, lcol * D:(lcol + 1) * D])
                        else:
                            nc.vector.tensor_reduce(
                                out=xl[:, D:2 * D],
                                in_=lg_t[:, lcol * D:(lcol + Kl) * D]
                                .rearrange("p (k f) -> p f k", k=Kl),
                                axis=mybir.AxisListType.X, op=ALU.add)
                        usrc = None
                        if Ku == 1:
                            usrc = ug_t[:, ucol * D:(ucol + 1) * D]
                        elif Ku > 1:
                            ub = wpool.tile([P, D], cdt, tag="ub")
                            nc.vector.tensor_reduce(
                                out=ub[:],
                                in_=ug_t[:, ucol * D:(ucol + Ku) * D]
                                .rearrange("p (k f) -> p f k", k=Ku),
                                axis=mybir.AxisListType.X, op=ALU.add)
                            usrc = ub[:]

                    pxl = pt_pool.tile([2 * D, P], cdt, tag="pxl")
                    nc.tensor.transpose(pxl[:], xl[:], ident[:])
                    xlT = wpool.tile([2 * D, P], cdt, tag="xlT")
                    nc.scalar.activation(xlT[:], pxl[:], AF.Copy)
                    uT = None
                    if usrc is not None:
                        ptu = pt_pool.tile([D, P], cdt, tag="ptu")
                        nc.tensor.transpose(ptu[:], usrc, ident[:])
                        uT = wpool.tile([D, P], cdt, tag="uT")
                        nc.scalar.activation(uT[:], ptu[:], AF.Copy)

                    ph = ph_pool.tile([P, D], f32)
                    nc.tensor.matmul(ph[:], xlT[:], w01[:],
                                     start=True, stop=(uT is None))
                    if uT is not None:
                        nc.tensor.matmul(ph[:], uT[:], w2[:],
                                         start=False, stop=True)

                    h = wpool.tile([P, D], cdt, tag="h")
                    nc.scalar.activation(h[:], ph[:], AF.Relu)
                    m = wpool.tile([P, P], cdt, tag="m")
                    nc.vector.tensor_scalar(
                        out=m[:], in0=iota[:], scalar1=batch[:, b:b + 1],
                        scalar2=None, op0=ALU.is_equal)
                    nc.tensor.matmul(pro[:], m[:], h[:],
                                     start=(b == 0), stop=(b == NB - 1))

                out_sb = wpool.tile([P, D], f32, tag="out")
                nc.scalar.activation(out_sb[:], pro[:], AF.Copy)
                nc.sync.dma_start(out_d[:], out_sb[:])

    nc.compile()
    return nc


# ---------------- top-level entry ----------------

def prepare(features, b1_rows, b1_cols, b1_vals, b2_rows, b2_cols, b2_vals,
            edge_batch, W0, W1, W2,
            n_nodes=N_NODES, n_edges=N_EDGES, n_tri=N_TRI, n_cores=N_CORES,
            gdt_name="float8e4"):
    """Host prep: returns (plan, nc, in_maps, counts). gdt_name sets the
    x/gather-table dtype (float8e4 or bfloat16); compute stays bf16."""
    import ml_dtypes
    features = np.asarray(features, np.float32)
    edge_batch = np.asarray(edge_batch, np.int64)
    lo_ptr, lo_e2, lo_sign, up_ptr, up_e2, up_sign = build_pairs(
        n_nodes, n_edges, n_tri, b1_rows, b1_cols, b1_vals,
        b2_rows, b2_cols, b2_vals)
    pl = make_plan(n_edges, n_cores, lo_ptr, up_ptr)

    np_xdt = (ml_dtypes.float8_e4m3 if gdt_name == "float8e4"
              else ml_dtypes.bfloat16)
    np_cdt = ml_dtypes.bfloat16
    xcast = features.astype(np_xdt)
    W0 = np.asarray(W0, np.float32); W1 = np.asarray(W1, np.float32)
    W2 = np.asarray(W2, np.float32)
    w01 = np.concatenate([W0 + 2.0 * W1, W1], axis=0).astype(np_cdt)  # [2D, D]
    w2_dev = W2.astype(np_cdt)

    w012 = np.concatenate([w01, w2_dev], axis=0)  # [3D, D]
    in_maps = []
    for c in range(n_cores):
        ci = build_core_inputs(pl, c, edge_batch,
                               lo_ptr, lo_e2, lo_sign, up_ptr, up_e2, up_sign)
        perm = pl.perms[c]
        xin = np.zeros((pl.NBP, D), np_xdt)
        xin[perm >= 0] = xcast[perm[perm >= 0]]
        bu8 = np.full((P, pl.NB), 255, np.uint8)
        real = ci["batchf"] >= 0
        bu8[real] = ci["batchf"][real].astype(np.uint8)
        meta = np.zeros((P, pl.META), np.uint8)
        for idx, off_lo, off_nib in ((ci["lidx"], pl.OFF_LLO, pl.OFF_LNIB),
                                     (ci["uidx"], pl.OFF_ULO, pl.OFF_UNIB)):
            w = idx.shape[1]
            lo = (idx & 0xFFFF).astype("<u2")
            hi = (idx >> 16).astype(np.uint8)
            meta[:, off_lo:off_lo + 2 * w] = lo.view(np.uint8)
            meta[:, off_nib:off_nib + w // 2] = hi[:, 0::2] | (hi[:, 1::2] << 4)
        meta[:, pl.OFF_BATCH:pl.OFF_BATCH + pl.NB] = bu8
        in_maps.append(dict(xin=xin, meta=meta, w012=w012))
    counts = np.bincount(edge_batch, minlength=G).astype(np.float32)
    nc = build_program(pl, xdt_name=gdt_name if gdt_name == "float8e4"
                       else "bfloat16")
    return pl, nc, in_maps, counts


def kernel(features, b1_rows, b1_cols, b1_vals, b2_rows, b2_cols, b2_vals,
           edge_batch, W0, W1, W2):
    from concourse.bass_utils import run_bass_kernel_spmd
    pl, nc, in_maps, counts = prepare(
        features, b1_rows, b1_cols, b1_vals, b2_rows, b2_cols, b2_vals,
        edge_batch, W0, W1, W2)
    res = None
    for attempt in range(3):
        try:
            res = run_bass_kernel_spmd(nc, in_maps, core_ids=list(range(N_CORES)))
            break
        except Exception:
            if attempt == 2:
                raise
    total = np.zeros((P, D), np.float32)
    for r in res.results:
        total += r["out"]
    g = total[:G] / np.maximum(counts, 1.0)[:, None]
    return (g, g.copy(), g.copy())
